# revision 1
# baseline (speedup 1.0000x reference)
"""AMIP router kernel for 8 TRN2 NeuronCores (Bass/Tile, SPMD data-parallel).

Strategy
--------
B*M = 2048 masked positions are sharded 256 per core (batch-major), weights
replicated, zero collectives.  Routing softmaxes / combine weights / gathers
and the small Bm = hm@W1b + b1 term (<4% of FLOPs combined) run on host; the
device runs the heavy expert MLPs over all 2560 tokens/core.

Since W2 is linear, the neighbor-window sum moves BEFORE matmul-2:

    G_i[h, m]   = wr_i[m] * sum_w cw[m, w] * gelu(ha_{m,w}@W1a_i + Bm_{m,i})
    delta^T     = sum_i W2_i^T @ G_i

so matmul-2 runs on 256 columns per (expert, ht) instead of 2560 -- 10x
less PE work than the baseline that accumulated all per-token products in
PSUM.  The expert-independent combine weight cw scales each token on DVE,
Pool (otherwise idle) accumulates the 10 w-groups into a [128, 512]
running sum (two w-parity halves), and DVE folds + scales by the router
weight wr_i into the [128, 256] matmul-2 rhs.  s = cw*wr factoring also
drops the per-expert broadcast of s from the input stream (-4.6MB DMA).

Layouts are feature-major ([feature_partition, token_free]) so both matmuls
chain without transposes.  Compute dtype bf16 (fp32 PSUM accumulate).

This walrus build enforces tiny per-instruction sync-wait budgets (DVE
tensor ops and 3-source activations: ONE wait; 2-source ACT copies and
matmuls: two; DMAs: one engine wait).  The kernel is choreographed to that
budget: per-engine program-order chaining via ordering-only dep edges, tiny
DVE "observer" copies that advance each engine's observed vector clock so
Tile elides all but one wait per op, all input tiles SBUF-resident, and a
patched kernel-tail drain split into single-wait drains.
"""

import sys

for _p in ("/opt/trn_rl_repo",):
    if _p not in sys.path:
        sys.path.insert(0, _p)

import numpy as np
import ml_dtypes

# Problem constants (hardcoded per task spec).
B, S, D, M, K, R = 4, 2048, 1024, 512, 8, 5
W = 2 * R                 # neighbor window size (10)
D4 = D // 4               # expert hidden (256)
NCORES = 8
MC = (B * M) // NCORES    # masked positions per core (256)
T = W * MC                # device tokens per core (2560), w-major order
NBLK = 5                  # 512-wide token blocks per h-tile (T/512)
CD = D // 128             # contraction chunks over D (8)
W1A_OFF, BM_OFF, W2_OFF, WR_OFF = 0, 2048, 3072, 5120
PK = WR_OFF + 256         # packed per-expert columns (5376)
BF16 = ml_dtypes.bfloat16

_COMPILED = {}            # cache: built Bass graph (shape-only, no data baked)
LAST_RESULT = None        # BassKernelResults of the most recent run
TRACE = False             # set True (e.g. from test.py) to profile


def _patch_tail_drain():
    """Split Tile's kernel-tail drain into several drains with <=4 sem waits
    each -- this walrus build rejects the single many-wait drain the stock
    _drain_and_barrier emits for a kernel touching all 8 HW DMA queues."""
    import concourse.tile as tile
    from concourse.vector_clock import ScopedClock, VectorClock

    if getattr(tile.TileContext, "_tail_drain_patched", False):
        return

    def _drain_and_barrier(self, tick_clock, wait_clock):
        g = tick_clock.global_clock
        n = len(g)
        ticks = [g[i] for i in range(n)]
        nz = [i for i, t in enumerate(ticks) if t > 0]
        CH = 1
        for j in range(0, len(nz), CH):
            keep = set(nz[j : j + CH])
            sub = VectorClock([ticks[i] if i in keep else 0 for i in range(n)])
            d = self.nc.sync.drain()
            wait_clock.add_sem_waits(d.ins, ScopedClock({None: sub}))
        if not nz:
            d = self.nc.sync.drain()
            wait_clock.add_sem_waits(
                d.ins, ScopedClock({None: tick_clock.global_clock})
            )
        self.nc.all_engine_barrier()
        assert self.sems is not None
        popped = self.nc._tile_sem_poison_stack.pop()
        assert popped is self._sem_poison
        self.nc.clear_and_free_semaphores(list(self.sems.allocated().values()))
        self.nc.all_engine_barrier()

    tile.TileContext._drain_and_barrier = _drain_and_barrier
    tile.TileContext._tail_drain_patched = True


def _build_nc():
    import concourse.bass as bass
    import concourse.mybir as mybir
    import concourse.tile as tile
    from contextlib import ExitStack

    _patch_tail_drain()

    bf = mybir.dt.bfloat16
    f32 = mybir.dt.float32
    AF = mybir.ActivationFunctionType

    nc = bass.Bass()
    # DRAM parameters (per-core shards; all pre-laid-out [partition, free]).
    xa = nc.declare_dram_parameter("xa", [128, CD, T], bf, isOutput=False)
    # packed per-expert: [w1a (CD*D4) | bm (2*512) | w2 (2*D) | wr (256)]
    wpk = nc.declare_dram_parameter("wpk", [K, 128, PK], bf, isOutput=False)
    # combine weights, single row, token (w-major) on free -- broadcast to
    # 128 partitions on-device by a ones-column matmul (5KB of DMA instead
    # of 640KB on the bandwidth-bound startup path)
    cwb = nc.declare_dram_parameter("cwb", [1, T], bf, isOutput=False)
    out = nc.declare_dram_parameter("out", [128, 8 * MC], f32, isOutput=True)

    with ExitStack() as ctx:
        tc = ctx.enter_context(tile.TileContext(nc))
        const = ctx.enter_context(tc.tile_pool(name="const", bufs=1))
        work = ctx.enter_context(tc.tile_pool(name="work", bufs=2))
        pd = ctx.enter_context(tc.tile_pool(name="pd", bufs=1, space="PSUM"))
        ph = ctx.enter_context(tc.tile_pool(name="ph", bufs=3, space="PSUM"))

        # Everything is resident in SBUF for the whole kernel -- no tile-slot
        # reuse for DMA'd inputs.  (Reused DMA slots create WAW deps against
        # the previous DMA's fanned-out HW queues, blowing the per-instruction
        # sync-wait slot budget in walrus.)
        # Per-engine program-order chaining (ordering-only edges): the
        # scheduler otherwise reorders ready instructions, which breaks the
        # carefully sequenced "observed clock" math that keeps every
        # instruction within its ISA struct's sync-wait budget.
        _last = {}

        def chain(instr, eng):
            if instr is None or not hasattr(instr, "ins"):
                return instr
            prev = _last.get(eng)
            if prev is not None:
                tile.add_dep_helper(
                    instr.ins, prev.ins, sync=False, reason="program-order"
                )
            _last[eng] = instr
            return instr

        # Stage xa: the first 512-token slice of every chunk lands first so
        # the first matmul block can start ~10us earlier; the tail follows.
        xa_sb = const.tile([128, CD, T], bf, tag="xa")
        cwb_sb = const.tile([1, T], bf, tag="cwb")
        nc.sync.dma_start(cwb_sb[:], cwb[:])
        nc.sync.dma_start(xa_sb[:, :, 0:512], xa[:, :, 0:512])
        # Explicit zero bias for Gelu: a float bias would be lowered to a
        # framework const AP whose init adds a second sync wait -- over the
        # 3-source Activation struct's budget of one.  DVE-owned zeros let
        # the bias dep consolidate with the DVE data dep into one wait.
        zcol = const.tile([128, 1], f32, tag="zcol")
        chain(nc.vector.memset(zcol[:], 0.0), "dve")
        # Self-chained ACT probe: waiting on its own semaphore advances the
        # scalar engine's observed self-clock, so each gelu's WAW wait
        # against the slot-recycled previous gelu is elided (the 3-source
        # Activation struct only has one sync-wait slot, needed for DVE).
        dummy_act = const.tile([1, 1], f32, tag="dummy_act")
        chain(nc.vector.memset(dummy_act[:], 0.0), "dve")
        # Warm the gelu activation-table load (~2.7us) during the input DMA
        # window instead of on the first real gelu.
        warm_t = const.tile([1, 1], f32, tag="warm_t")
        chain(
            nc.scalar.activation(
                warm_t[:], zcol[0:1, :], AF.Gelu, bias=zcol[0:1, :]
            ),
            "act",
        )
        # DVE observer scratch: tiny copies that advance VectorE's observed
        # clocks of other engines so real DVE ops carry a single sync wait
        # (this walrus build allows only ONE wait on DVE TT/Copy structs).
        scr1 = const.tile([1, 1], bf, tag="scr1")
        scr2 = const.tile([1, 512], bf, tag="scr2")
        scr2e = const.tile([1, 512], bf, tag="scr2e")
        scrp = const.tile([1, 1], bf, tag="scrp")
        # PE warm-up: ~20 rank-1 matmuls (~5us of PE activity) during the
        # input-DMA window keep the HAM clock gate from starting the real
        # matmul stream at half rate.  Dedicated source tile so no real
        # consumer inherits a WAR dep against the warm matmuls.
        warm_src = const.tile([1, 512], bf, tag="warm_src")
        chain(nc.vector.memset(warm_src[:], 0.0), "dve")
        ones_t = const.tile([1, 128], bf, tag="ones_t")
        chain(nc.vector.memset(ones_t[:], 1.0), "dve")
        warm_ps = pd.tile([128, 512], f32, tag="warm_ps", name="warm_ps")
        for wk in range(14):
            chain(nc.tensor.matmul(
                warm_ps[:],
                warm_src[0:1, 0:128],
                warm_src[0:1, :],
                start=(wk == 0),
                stop=(wk == 13),
                skip_group_check=True,
            ), "pe")

        # PE "touch" matmuls: rank-1 reads of a freshly DMA'd region that
        # carry the DMA-queue wait on a throwaway instruction, advancing the
        # PE's observed queue clock so the real matmuls (which also need a
        # DVE slot-WAR wait) stay within the single-wait Matmult budget.
        # They overwrite a corner of the (finished) warm-up bank -- a
        # dedicated PSUM tile would cost the bank that hid_ps triple
        # buffering needs.
        def touch(region):
            chain(nc.tensor.matmul(
                warm_ps[0:1, 0:1],
                region,
                region,
                start=True,
                stop=True,
                skip_group_check=True,
            ), "pe")

        # Input stream ordered by first-use time on the critical path: the
        # aggregate DMA bandwidth is the startup gate, so late-needed chunks
        # (cwb tail, wr, w2, experts 1-7) queue after the xa slices expert 0
        # consumes in its first two groups.
        wpk_all = const.tile([128, K, PK], bf, tag="wpk_all")

        def wpk0(lo, hi):
            nc.sync.dma_start(wpk_all[:, 0, lo:hi], wpk[0, :, lo:hi])

        def xa_slice(blk):
            nc.sync.dma_start(
                xa_sb[:, :, blk * 512 : blk * 512 + 512],
                xa[:, :, blk * 512 : blk * 512 + 512],
            )

        wpk0(W1A_OFF, W1A_OFF + 1024)
        wpk0(BM_OFF, W2_OFF)
        xa_slice(1)
        wpk0(W1A_OFF + 1024, BM_OFF)
        xa_slice(2)
        xa_slice(3)
        xa_slice(4)
        wpk0(WR_OFF, PK)
        # Experts 1-7 split by first-use: [w1a+bm | wr | w2] -- with the
        # expert-0 groups interleaved, the PE reaches expert 1 at ~21us,
        # before a monolithic expert-1 transfer could land.  The w2 chunks
        # trail by an expert (first needed at the NEXT expert's mm2 slot).
        nc.sync.dma_start(wpk_all[:, 1, 0:W2_OFF], wpk[1, :, 0:W2_OFF])
        nc.sync.dma_start(wpk_all[:, 1, WR_OFF:PK], wpk[1, :, WR_OFF:PK])
        wpk0(W2_OFF, WR_OFF)
        for i in range(2, K):
            nc.sync.dma_start(wpk_all[:, i, 0:W2_OFF], wpk[i, :, 0:W2_OFF])
            nc.sync.dma_start(wpk_all[:, i, WR_OFF:PK], wpk[i, :, WR_OFF:PK])
            nc.sync.dma_start(
                wpk_all[:, i - 1, W2_OFF:WR_OFF], wpk[i - 1, :, W2_OFF:WR_OFF]
            )
        nc.sync.dma_start(wpk_all[:, K - 1, W2_OFF:WR_OFF], wpk[K - 1, :, W2_OFF:WR_OFF])

        # Broadcast combine weights to all partitions via ones-column
        # matmuls (in the idle slot right after warm-up), then ACT stages
        # them into per-block tiles: DVE instructions cannot carry a
        # DMA-queue wait in this walrus build, so every DVE-read tile
        # needs an ACT/DVE/PE-observable producer.
        cw_st = const.tile([128, NBLK, 512], bf, tag="cw_st")
        touch(cwb_sb[0:1, 0:1])
        for blk in range(NBLK):
            bc_ps = ph.tile([128, 512], f32, tag="hid")
            chain(nc.tensor.matmul(
                bc_ps[:],
                ones_t[:],
                cwb_sb[0:1, blk * 512 : blk * 512 + 512],
                start=True,
                stop=True,
                skip_group_check=True,
            ), "pe")
            chain(nc.scalar.copy(cw_st[:, blk, :], bc_ps[:]), "act")

        # Output accumulator in PSUM: delta^T [1024, 256] as 4 banks of
        # [128, 512], each holding two 128-row d-chunks side by side.
        delta_ps = [
            pd.tile([128, 512], f32, tag=f"d{j}", name=f"delta_ps{j}")
            for j in range(4)
        ]

        # Software-pipeline matmul2 one group behind so the PE never
        # head-of-line blocks on the V->S->V->Pool reduction chain.
        pending = []  # [(i, ht, G_tile)]

        def emit_mm2(p):
            i_, ht_, g_ = p
            w2_p = wpk_all[:, i_]
            first = i_ == 0 and ht_ == 0
            last = i_ == K - 1 and ht_ == 1
            for dt in range(8):
                sl = delta_ps[dt // 2][:, (dt % 2) * 256 : (dt % 2) * 256 + 256]
                # start=True clears has_written for the WHOLE BANK, and
                # each bank holds two dt regions -- so only the first
                # region of each bank may issue start.  The second
                # region's first write overwrites (bits cleared by the
                # bank's single start) and accumulates thereafter.
                chain(nc.tensor.matmul(
                    sl,
                    w2_p[:, W2_OFF + ht_ * D + dt * 128 : W2_OFF + ht_ * D + dt * 128 + 128],
                    g_[:],
                    start=(first and dt % 2 == 0),
                    stop=last,
                    skip_group_check=True,
                ), "pe")

        hw_hist = []              # all hw tiles in DVE emission order
        pb_prev = [None]
        deferred_gt = []          # [(g1, wr, i, ht)] Pool reduce done, G not yet
        first_of_expert = [False]

        def emit_gt():
            """G = g1 * wr on DVE, deferred into the NEXT group's stream:
            emitted in the group tail it would make the in-order DVE queue
            wait ~2.5us for the Pool chain, stalling the next group's
            bias-adds (and transitively the PE via the hid-slot WAR)."""
            g1_, wr_, i_, ht_ = deferred_gt.pop(0)
            g_t = work.tile([128, 256], bf, tag="G")
            chain(nc.vector.tensor_mul(g_t[:], g1_[:], wr_[:]), "dve")
            pending.append((i_, ht_, g_t))

        def new_group(i, ht, wp, bm_sb, wr_sb):
            return dict(i=i, ht=ht, wp=wp, bm=bm_sb, wr=wr_sb,
                        hw_g=[], pend_mul=[], e_pair=[],
                        pool_reduce=not (i == K - 1 and ht == 1),
                        pa=None, pb=None)

        def emit_mul(g):
            """mul for the group's oldest gelu'd block.  Runs one block
            behind the bias-add so the V->S->V round-trip (gelu + two sem
            hops, ~1us) overlaps the next bias-add instead of serializing
            the per-block pipeline.  The mul carries the gelu wait itself
            (its only sem wait), which also advances DVE's observed ACT
            clock for the next bias-add."""
            tmp_, blk_ = g["pend_mul"].pop(0)
            hw_t = work.tile([128, 512], bf, tag="hw", bufs=10)
            chain(nc.vector.tensor_mul(hw_t[:], tmp_[:], cw_st[:, blk_, :]), "dve")
            g["hw_g"].append(hw_t)
            hw_hist.append(hw_t)
            n = len(g["hw_g"])
            # Pool (otherwise idle) owns the w-window reduction as a TREE:
            # pair-adds of DVE products carry one DVE wait; the self-chained
            # combines carry one Pool self-wait.
            if n == 2 and g["pool_reduce"]:
                pa_ = work.tile([128, 512], bf, tag="pa")
                chain(nc.gpsimd.tensor_add(pa_[:], g["hw_g"][0][:], g["hw_g"][1][:]), "pool")
                g["pa"] = pa_
            elif n == 2:
                e1_ = work.tile([128, 512], bf, tag="e1", bufs=1)
                chain(nc.vector.tensor_add(e1_[:], g["hw_g"][0][:], g["hw_g"][1][:]), "dve")
                g["e_pair"].append(e1_)
            elif n == 4 and not g["pool_reduce"]:
                e2_ = work.tile([128, 512], bf, tag="e2", bufs=1)
                chain(nc.vector.tensor_add(e2_[:], g["hw_g"][2][:], g["hw_g"][3][:]), "dve")
                # pre-fold everything not depending on the last block:
                # e4 = fold(e1 + e2) -- the post-mul(b4) chain shrinks to
                # mul, fold(hw4), add, scale.
                e3_ = work.tile([128, 512], bf, tag="e3", bufs=1)
                chain(nc.vector.tensor_add(e3_[:], g["e_pair"][0][:], e2_[:]), "dve")
                e4_ = work.tile([128, 256], bf, tag="e4", bufs=1)
                chain(nc.vector.tensor_add(e4_[:], e3_[:, 0:256], e3_[:, 256:512]), "dve")
                g["e_pair"].append(e4_)
            return hw_t

        def step(g, blk):
            i, ht = g["i"], g["ht"]
            wp = g["wp"]
            # absorb DMA first-touch waits on throwaway touch matmuls
            if i == 0 and ht == 0:
                touch(xa_sb[0:1, 0, blk * 512 : blk * 512 + 1])
            if blk == 0:
                if i == 0:
                    touch(wp[0:1, W1A_OFF + ht * 1024 : W1A_OFF + ht * 1024 + 1])
                elif ht == 0:
                    touch(wp[0:1, 0:1])
            hid_ps = ph.tile([128, 512], f32, tag="hid")
            for c in range(CD):
                chain(nc.tensor.matmul(
                    hid_ps[:],
                    wp[:, W1A_OFF + ht * 1024 + c * 128 : W1A_OFF + ht * 1024 + c * 128 + 128],
                    xa_sb[:, c, blk * 512 : blk * 512 + 512],
                    start=(c == 0),
                    stop=(c == CD - 1),
                    skip_group_check=True,
                ), "pe")
            if blk in (3, 4) and pending:
                # one-plus groups behind: the reduction chain finishes
                # ~4.2us after a group's last mm1, so emitting here keeps
                # the PE stall-free (a stall resets the p-state ramp).
                p = pending.pop(0)
                # every expert's w2 arrives as a separate DMA chunk; its
                # queue wait rides a touch, not the matmul.
                touch(wpk_all[0:1, p[0], W2_OFF : W2_OFF + 1])
                emit_mm2(p)
            # obs0: a DVE self-wait on the latest mult advances the
            # observed self-clock, eliding every older same-engine WAW/RAW
            # (recycled tmp/scr2 slots etc).
            if hw_hist:
                chain(nc.vector.tensor_copy(scr1[:], hw_hist[-1][0:1, 0:1]), "dve")
            # obs2-early at expert start: the bias-add below reads this
            # expert's freshly ACT-staged bm tile.  Writes its own scratch
            # -- sharing scr2 with other observers would add a WAR wait.
            if first_of_expert[0]:
                chain(nc.vector.tensor_copy(scr2e[:], g["bm"][0:1, 0:512]), "dve")
                first_of_expert[0] = False
            if blk == 4 and ht == 0:
                # Stage the router-weight row here: its only reader is the
                # deferred g_t in the next group, and staging it at expert
                # start would park the in-order ACT queue on the late-
                # arriving wr DMA chunk, stalling every gelu behind it.
                # Before this block's gelu, so the b4 mul's ACT wait covers
                # it for the g_t that fires before the next group's muls.
                chain(nc.scalar.copy(g["wr"][:], g["wp"][:, WR_OFF : WR_OFF + 256]), "act")
            tmp = work.tile([128, 512], bf, tag="tmp", bufs=3)
            chain(nc.vector.tensor_add(tmp[:], hid_ps[:], g["bm"][:, ht * 512 : ht * 512 + 512]), "dve")
            # probe: glues ACT to this iteration; its DVE wait makes the
            # in-place gelu need no further waits.
            chain(nc.scalar.mul(dummy_act[:], tmp[0:1, 0:1], 0.0), "act")
            chain(nc.scalar.activation(tmp[:], tmp[:], AF.Gelu, bias=zcol[:]), "act")
            g["pend_mul"].append((tmp, blk))
            if len(g["pend_mul"]) == 2:
                if blk in (1, 2) and deferred_gt:
                    emit_gt()
                hw_new = emit_mul(g)
                if blk == 4 and g["pool_reduce"]:
                    p_b = work.tile([128, 512], bf, tag="pb")
                    chain(nc.gpsimd.tensor_add(p_b[:], g["hw_g"][2][:], hw_new[:]), "pool")
                    g["pb"] = p_b

        def tail(g):
            # the final block's mul, then the combine chain.
            hw_last = emit_mul(g)
            if not g["pool_reduce"]:
                # Final group: all-DVE tree, everything but the last block
                # pre-folded into e4 -- only ~0.9us of chain remains after
                # mul(b4), so the last mm2 starts ~2.5us sooner than via
                # the (possibly queued) Pool path.
                q1 = work.tile([128, 256], bf, tag="g1")
                chain(nc.vector.tensor_add(q1[:], hw_last[:, 0:256], hw_last[:, 256:512]), "dve")
                g1 = work.tile([128, 256], bf, tag="g1")
                chain(nc.vector.tensor_add(g1[:], q1[:], g["e_pair"][1][:]), "dve")
                g_t = work.tile([128, 256], bf, tag="G")
                chain(nc.vector.tensor_mul(g_t[:], g1[:], g["wr"][:]), "dve")
                pending.append((g["i"], g["ht"], g_t))
            else:
                # f1 = pa + pb; f2 = f1 + hw4; g1 folds the two w-parity
                # halves -- all Pool, each one self/DVE wait.
                f1 = work.tile([128, 512], bf, tag="f1")
                chain(nc.gpsimd.tensor_add(f1[:], g["pa"][:], g["pb"][:]), "pool")
                # second Pool self-observer: f2 reads f1 (self) AND hw4
                # (DVE) -- two waits without this; observing f1 here leaves
                # f2 with only the DVE data wait.
                chain(nc.gpsimd.tensor_copy(scrp[:], f1[0:1, 0:1]), "pool")
                f2 = work.tile([128, 512], bf, tag="f2")
                chain(nc.gpsimd.tensor_add(f2[:], f1[:], hw_last[:]), "pool")
                g1 = work.tile([128, 256], bf, tag="g1")
                chain(nc.gpsimd.tensor_add(g1[:], f2[:, 0:256], f2[:, 256:512]), "pool")
                deferred_gt.append((g1, g["wr"], g["i"], g["ht"]))
                pb_prev[0] = g["pb"]

        def obs_p():
            # Pool self-observer: one Pool self-wait on the last group's
            # final reduce advances Pool's observed self-clock, so the
            # next pair-adds carry only their DVE data wait (the recycled-
            # slot WAW would otherwise be a second wait).
            if pb_prev[0] is not None:
                chain(nc.gpsimd.tensor_copy(scrp[:], pb_prev[0][0:1, 0:1]), "pool")

        for i in range(K):
            wp = wpk_all[:, i]
            # Bm_i (= hm @ W1b_i + b1_i, already w-replicated) is computed
            # on host (~3% of FLOPs) and staged via ScalarE so the DVE add
            # sees an ACT producer (single-wait budget on DVE TT ops).
            # Same for the router-weight row wr_i.
            bm_sb = work.tile([128, 1024], bf, tag="bm_sb")
            chain(nc.scalar.copy(bm_sb[:], wp[:, BM_OFF : BM_OFF + 1024]), "act")
            wr_sb = work.tile([128, 256], bf, tag="wr_sb")  # staged in tail(ht0)
            first_of_expert[0] = True
            if i == 0:
                # Expert 0 is input-DMA gated: interleave its two ht-groups
                # so each freshly landed xa slice feeds TWO mm1 blocks --
                # PE demand then matches the DMA arrival rate and the
                # p-state ramp never resets.
                ga = new_group(0, 0, wp, bm_sb, wr_sb)
                gb = new_group(0, 1, wp, bm_sb, wr_sb)
                obs_p()
                for blk in range(NBLK):
                    step(ga, blk)
                    step(gb, blk)
                tail(ga)
                tail(gb)
            else:
                for ht in range(2):
                    g = new_group(i, ht, wp, bm_sb, wr_sb)
                    obs_p()
                    for blk in range(NBLK):
                        step(g, blk)
                    tail(g)
        # Ramp-keeper: the PE would otherwise idle ~2us here waiting for the
        # final group's reduction, resetting the p-state ramp and running
        # the last mm2 at half clock.  Filler matmuls (no data deps) keep
        # it busy and ramped; sized to finish just as G lands.
        for wk in range(8):
            chain(nc.tensor.matmul(
                warm_ps[:],
                warm_src[0:1, 0:128],
                warm_src[0:1, :],
                start=(wk == 0),
                stop=(wk == 7),
                skip_group_check=True,
            ), "pe")

        # Final group's mm2 interleaved with the PSUM -> SBUF -> DRAM drain:
        # as soon as a delta bank receives its last accumulation, ACT/DVE
        # copy it out and SWDGE ships it while the PE fills the next bank.
        (i_, ht_, g_) = pending.pop(0)
        assert not pending
        w2_p = wpk_all[:, i_]
        delta_sb = const.tile([128, 4, 512], f32, tag="dsb")
        for j in range(4):
            for h in range(2):
                dt = 2 * j + h
                chain(nc.tensor.matmul(
                    delta_ps[j][:, h * 256 : h * 256 + 256],
                    w2_p[:, W2_OFF + ht_ * D + dt * 128 : W2_OFF + ht_ * D + dt * 128 + 128],
                    g_[:],
                    start=False,
                    stop=True,
                    skip_group_check=True,
                ), "pe")
            if j % 2 == 0:
                chain(nc.scalar.copy(delta_sb[:, j, :], delta_ps[j][:]), "act")
            else:
                chain(
                    nc.vector.tensor_copy(delta_sb[:, j, :], delta_ps[j][:]),
                    "dve",
                )
            # SWDGE for outputs: the HW queues all have prior traffic, and a
            # queue-FIFO self-wait + the ACT data wait exceeds the DMA
            # struct's single sync-wait slot.
            chain(nc.gpsimd.dma_start(out[:, j * 512 : j * 512 + 512], delta_sb[:, j, :]), "pool")
    return nc


def _softmax(x, axis=-1):
    x = x - x.max(axis=axis, keepdims=True)
    e = np.exp(x)
    return e / e.sum(axis=axis, keepdims=True)


def kernel(h_L, mask_indices, unmasked_indices, range_r, Wr, br, W1, b1, W2, b2):
    global LAST_RESULT
    from concourse.bass_utils import run_bass_kernel_spmd

    h_L = np.asarray(h_L, np.float32)
    mask_indices = np.asarray(mask_indices, np.int32)
    unmasked_indices = np.asarray(unmasked_indices, np.int32)
    Wr, br = np.asarray(Wr, np.float32), np.asarray(br, np.float32)
    W1, b1 = np.asarray(W1, np.float32), np.asarray(b1, np.float32)
    W2, b2 = np.asarray(W2, np.float32), np.asarray(b2, np.float32)
    assert int(range_r) == R and h_L.shape == (B, S, D)

    # ---- host: gathers, masks, routing/combine softmaxes ----
    offs = np.concatenate([np.arange(-R, 0), np.arange(1, R + 1)])  # [W]
    a = mask_indices                                                # [B,M]
    t = a[:, :, None] + offs[None, None, :]                         # [B,M,W]
    in_range = (t >= 0) & (t < S)
    tcl = np.clip(t, 0, S - 1)
    is_un = np.zeros((B, S), bool)
    is_un[np.arange(B)[:, None], unmasked_indices] = True
    valid = in_range & is_un[np.arange(B)[:, None, None], tcl]      # [B,M,W]

    bidx = np.arange(B)[:, None]
    h_mask = h_L[bidx, a]                                           # [B,M,D]
    h_anchor = h_L[np.arange(B)[:, None, None], tcl]                # [B,M,W,D]

    wr = _softmax(h_mask @ Wr + br, axis=-1)                        # [B,M,K]
    scores = np.einsum("bmwd,bmd->bmw", h_anchor, h_mask) / np.sqrt(
        np.float32(D)
    )
    scores = np.where(valid, scores, np.float32(-1e30))
    cw = _softmax(scores, axis=-1) * valid                          # [B,M,W]

    # ---- build per-core shards ----
    # W1 split + pre-transposed chunk layouts.
    W1a = W1[:, :D, :]                                              # [K,D,D4]
    W1b = W1[:, D:, :]
    # ht-major columns (ht*1024 + c*128 + h) so expert-0's first h-tile
    # needs only the first half of the w1a transfer.
    w1a_l = np.ascontiguousarray(
        W1a.reshape(K, CD, 128, 2, 128).transpose(0, 2, 3, 1, 4)
    ).astype(BF16)                                                  # [K,128,2,CD,128]
    w2_l = np.ascontiguousarray(
        W2.reshape(K, 2, 128, D).transpose(0, 2, 1, 3)
    ).astype(BF16)                                                  # [K,128,2,D]
    # Bm = h_mask @ W1b + b1 computed on host (~3% of FLOPs), saves device
    # matmuls.  [B,M,K,D4]
    Bm_h = np.einsum("bmd,kdh->bmkh", h_mask, W1b) + b1[None, None]

    in_maps = []
    for c in range(NCORES):
        b = c // 2
        ms = (c % 2) * MC
        ha_c = h_anchor[b, ms : ms + MC]                            # [MC,W,D]
        # tokens w-major: [W,MC,D] -> [T,D] -> transpose [D,T]
        xaT = ha_c.transpose(1, 0, 2).reshape(T, D).T               # [D,T]
        xa_l = np.ascontiguousarray(
            xaT.reshape(CD, 128, T).transpose(1, 0, 2)
        ).astype(BF16)                                              # [128,CD,T]
        # Bm^T per expert/h-tile, replicated x2 along free to match the
        # 512-wide (two w-group) blocks: [K,128,2,512]
        bm_c = Bm_h[b, ms : ms + MC]                                # [MC,K,D4]
        bmT = bm_c.transpose(1, 2, 0).reshape(K, 2, 128, MC)        # [K,ht,128,MC]
        bm_l = np.ascontiguousarray(
            np.broadcast_to(
                bmT.transpose(0, 2, 1, 3)[:, :, :, None, :],
                (K, 128, 2, 2, MC),
            ).reshape(K, 128, 2, 512)
        ).astype(BF16)
        # router weights per expert, partition-broadcast: [K,128,256]
        wr_c = wr[b, ms : ms + MC]                                  # [MC,K]
        wr_l = np.broadcast_to(
            wr_c.T[:, None, :], (K, 128, MC)
        ).astype(BF16)
        # combine weights w-major, single row (device broadcasts): [1, T]
        cw_c = cw[b, ms : ms + MC]                                  # [MC,W]
        cwb_l = cw_c.T.reshape(1, T).astype(BF16)
        wpk_l = np.concatenate(
            [
                w1a_l.reshape(K, 128, CD * D4),
                bm_l.reshape(K, 128, 1024),
                w2_l.reshape(K, 128, 2 * D),
                wr_l,
            ],
            axis=2,
        )
        in_maps.append(dict(xa=xa_l, wpk=wpk_l, cwb=np.ascontiguousarray(cwb_l)))

    key = "nc"
    if key not in _COMPILED:
        _COMPILED[key] = _build_nc()
    nc = _COMPILED[key]

    res = run_bass_kernel_spmd(
        nc, in_maps, core_ids=list(range(NCORES)), trace=TRACE
    )
    LAST_RESULT = res

    # ---- host: unshard + b2 correction + scatter ----
    delta_h = np.zeros((B, S, D), np.float32)
    sw = cw.sum(-1)                                                 # [B,M]
    corr = (sw[:, :, None] * (wr @ b2)).astype(np.float32)          # [B,M,D]
    for c in range(NCORES):
        b = c // 2
        ms = (c % 2) * MC
        o = res.results[c]["out"]                                   # [128, 8*MC]
        dT = o.reshape(128, 8, MC).transpose(1, 0, 2).reshape(D, MC)
        delta = dT.T + corr[b, ms : ms + MC]                        # [MC,D]
        delta_h[b, a[b, ms : ms + MC]] = delta
    return delta_h

